# revision 1
# baseline (speedup 1.0000x reference)
"""AllTripletLoss Trainium2 kernel (8-core SPMD, Bass/Tile).

Algorithm (matches reference.py):
    sim = X @ X.T                       [n, n], n=8192, d=128
    pos_mask = same-class & ~eye ; neg_mask = ~same-class
    max_pos = rowmax(sim | pos_mask) ; max_neg = rowmax(sim | neg_mask)
    sel_pos = pos_mask & (sim < max_neg + 0.2)
    sel_neg = neg_mask & (sim > max(0.6, max_pos) - 0.2)
    loss = sum_rows(has_pos ? sum(sel_pos*(1-sim)) + sum(sel_neg*sim) : 0) / n
    neg_count = #rows(any(sel_neg) & has_pos)

Strategy:
  * Host: stable-sort rows by class so each row's positives occupy ONE
    contiguous column band [lo_i, hi_i).  Upload X_sorted^T once per core.
  * Rows sharded 1024/core.  Per 128-row m-tile the union of bands fits a
    static 384-col "zone" (class count <= ~40 << 128), so all mask work
    happens on a [128, 384] tile; everything outside the band is negative.
  * max_neg = max( blockwise 128-col maxes with the 3 zone blocks masked,
    zone max with in-band cols masked ).  Masks are additive {0, -3000}
    so kept values stay bit-exact.
  * Selection sums over the full row reduce to two ScalarE passes reading
    PSUM directly (sim is recomputed by a second matmul sweep, never
    materialized in SBUF/HBM):
      relu_sum = sum relu(sim - thr_n)        (activation Relu, accum_out)
      cnt      = (sum sign(sim - thr_n) + n)/2 (activation Sign, accum_out)
    then  sum_{sim>thr} sim = relu_sum + thr_n * cnt,  minus the in-band
    contribution recomputed exactly on the zone tile.
  * Schedule: per 128-row m-tile, phase A (matmul sweep 1 + PSUM block-max
    on VectorE + zone threshold chain) is software-pipelined one m-tile
    ahead of phase B (matmul sweep 2 + the two ScalarE accum passes), with
    A/B PSUM tiles interleaved through one shared 2-slot x 4-bank pool so
    ScalarE (the bottleneck at ~135us/core) stays saturated.
  * Per-core partial [sum(row_loss), neg_count] -> host reduces 8 pairs.
  * Engines/core (cost model): ACT 135us, DVE 117us, PE 60us, Pool 40us;
    span ~182us (matches HW measurement).
"""

from contextlib import ExitStack

import numpy as np

import concourse.bass as bass
import concourse.bacc as bacc
import concourse.tile as tile
from concourse import mybir
from concourse.bass_utils import run_bass_kernel_spmd

N = 8192
D = 128
NCORES = 8
RPC = N // NCORES          # rows per core
P = 128                    # partitions / m-tile rows
MT = RPC // P              # m-tiles per core
NCH = 16                   # 512-col chunks per row
CH = 512
NBLK = N // P              # 64 column blocks of 128
ZW = 384                   # zone width (3 blocks)
BIG = 3000.0               # additive exclusion mask magnitude
MARGIN = 0.2
NEG_FLOOR = 0.6

f32 = mybir.dt.float32
f32r = mybir.dt.float32r
ALU = mybir.AluOpType
ACTF = mybir.ActivationFunctionType


def build_nc(bench_reps: int = 0) -> bass.Bass:
    nc = bacc.Bacc("TRN2", target_bir_lowering=False)

    BW_ = N + RPC + MT * ZW
    MW_ = 2 * ZW + 1
    big_d = nc.dram_tensor("bigin", [D, BW_], f32r, kind="ExternalInput")
    mz_d = nc.dram_tensor("maskz", [MT, P, MW_], f32, kind="ExternalInput")
    adz_d = nc.dram_tensor("addz", [MT, NBLK], f32, kind="ExternalInput")
    out_d = nc.dram_tensor("out", [1, 2], f32, kind="ExternalOutput")

    with tile.TileContext(nc) as tc, ExitStack() as ctx:
        consts = ctx.enter_context(tc.tile_pool(name="consts", bufs=1))
        zaux = ctx.enter_context(tc.tile_pool(name="zaux", bufs=6))
        scr = ctx.enter_context(tc.tile_pool(name="scr", bufs=1))
        zwork = ctx.enter_context(tc.tile_pool(name="zwork", bufs=4))
        small = ctx.enter_context(tc.tile_pool(name="small", bufs=8))
        psum = ctx.enter_context(tc.tile_pool(name="pp", bufs=2, space="PSUM"))

        PP = 4 * CH               # 2048-wide psum tiles (4 banks)
        HH = NCH // 4             # 4 tiles per m-tile row
        LAG = 1

        big_sb = consts.tile([D, BW_], f32r)
        # split the load, critical-path first: xtm + first zones, then xt
        # quarters in consumption order, then the remaining zones
        Q_ = N // 4
        segs = [(N, N + RPC + 2 * ZW), (0, Q_), (Q_, 2 * Q_),
                (N + RPC + 2 * ZW, BW_), (2 * Q_, 3 * Q_), (3 * Q_, N)]
        for a_, b_ in segs:
            nc.gpsimd.dma_start(out=big_sb[:, a_:b_], in_=big_d[:, a_:b_])
        xt_sb = big_sb[:, 0:N]
        xtm_sb = big_sb[:, N:N + RPC]

        # addz is per-(m-tile, block); broadcast over the 128 partitions.
        adz_sb = consts.tile([P, MT, NBLK], f32)
        adz_ap = adz_d[:, :]
        nc.gpsimd.dma_start(
            out=adz_sb,
            in_=bass.AP(
                tensor=adz_ap.tensor,
                offset=adz_ap.offset,
                ap=[[0, P]] + [list(p) for p in adz_ap.ap],
            ),
        )

        accpair = consts.tile([P, 2], f32)
        junk = scr.tile([P, PP], f32)

        st = [{} for _ in range(MT)]

        def a_setup(mt):
            s = st[mt]
            lhs = xtm_sb[:, mt * P:(mt + 1) * P]
            xtz_sb = big_sb[:, N + RPC + mt * ZW:N + RPC + (mt + 1) * ZW]
            mz_sb = zaux.tile([P, MW_], f32, tag="mz")
            nc.gpsimd.dma_start(out=mz_sb, in_=mz_d[mt, :, :])
            s["hp"] = mz_sb[:, 2 * ZW:2 * ZW + 1]
            s["mz"] = mz_sb

            pz = psum.tile([P, PP], f32, tag="pp")
            nc.tensor.matmul(pz[:, 0:ZW], lhs, xtz_sb, start=True, stop=True)
            simz = zwork.tile([P, ZW], f32, tag="simz")
            nc.vector.tensor_copy(simz, pz[:, 0:ZW])
            s["simz"] = simz
            s["B"] = zwork.tile([P, NBLK], f32, tag="B", name="Btile")

        def a_tile(mt, h):
            s = st[mt]
            lhs = xtm_sb[:, mt * P:(mt + 1) * P]
            pa = psum.tile([P, PP], f32, tag="pp")
            for q_ in range(4):
                c0 = (4 * h + q_) * CH
                nc.tensor.matmul(
                    pa[:, q_ * CH:(q_ + 1) * CH], lhs,
                    xt_sb[:, c0:c0 + CH], start=True, stop=True)
            nc.vector.tensor_reduce(
                out=s["B"][:, h * 16:(h + 1) * 16],
                in_=pa.rearrange("p (b w) -> p b w", w=P),
                axis=mybir.AxisListType.X,
                op=ALU.max)

        def a_chain(mt):
            s = st[mt]
            mz_sb = s["mz"]
            inb_sb = mz_sb[:, 0:ZW]
            pos_sb = mz_sb[:, ZW:2 * ZW]
            simz = s["simz"]
            B = s["B"]

            junkB = zwork.tile([P, NBLK], f32, tag="junkB")
            bneg = small.tile([P, 1], f32)
            nc.vector.tensor_add(junkB, B, adz_sb[:, mt, :])
            nc.vector.tensor_reduce(
                out=bneg, in_=junkB, axis=mybir.AxisListType.X, op=ALU.max)
            zaddn = zwork.tile([P, ZW], f32, tag="zaddn")
            nc.vector.tensor_scalar_mul(zaddn, inb_sb, -BIG)
            zjunk = zwork.tile([P, ZW], f32, tag="zjunk")
            zneg = small.tile([P, 1], f32)
            nc.vector.tensor_add(zjunk, simz, zaddn)
            nc.vector.tensor_reduce(
                out=zneg, in_=zjunk, axis=mybir.AxisListType.X, op=ALU.max)
            maxneg = small.tile([P, 1], f32)
            nc.vector.tensor_max(maxneg, bneg, zneg)

            zaddp = zwork.tile([P, ZW], f32, tag="zaddp")
            nc.vector.tensor_scalar(
                out=zaddp, in0=pos_sb, scalar1=-1.0, scalar2=BIG,
                op0=ALU.add, op1=ALU.mult)
            zjunk2 = zwork.tile([P, ZW], f32, tag="zjunk2")
            maxpos = small.tile([P, 1], f32)
            nc.vector.tensor_add(zjunk2, simz, zaddp)
            nc.vector.tensor_reduce(
                out=maxpos, in_=zjunk2, axis=mybir.AxisListType.X, op=ALU.max)

            thrp = small.tile([P, 1], f32)
            nc.vector.tensor_scalar_add(thrp, maxneg, MARGIN)
            thrn = small.tile([P, 1], f32)
            nc.vector.tensor_scalar(
                out=thrn, in0=maxpos, scalar1=NEG_FLOOR, scalar2=-MARGIN,
                op0=ALU.max, op1=ALU.add)
            nthrn = small.tile([P, 1], f32)
            nc.vector.tensor_scalar_mul(nthrn, thrn, -1.0)
            s["thrn"], s["nthrn"] = thrn, nthrn

            cmp01 = zwork.tile([P, ZW], f32, tag="cmp01")
            nc.vector.tensor_scalar(
                out=cmp01, in0=simz, scalar1=thrp, scalar2=None, op0=ALU.is_lt)
            selp = zwork.tile([P, ZW], f32, tag="selp")
            cntp = small.tile([P, 1], f32)
            nc.gpsimd.tensor_mul(selp, cmp01, pos_sb)
            nc.vector.tensor_reduce(
                out=cntp, in_=selp, axis=mybir.AxisListType.X, op=ALU.add)
            zjunk3 = zwork.tile([P, ZW], f32, tag="zjunk3")
            spsum = small.tile([P, 1], f32)
            nc.gpsimd.tensor_mul(zjunk3, selp, simz)
            nc.vector.tensor_reduce(
                out=spsum, in_=zjunk3, axis=mybir.AxisListType.X, op=ALU.add)
            s["cntp"], s["spsum"] = cntp, spsum

            g01 = zwork.tile([P, ZW], f32, tag="g01")
            nc.vector.tensor_scalar(
                out=g01, in0=simz, scalar1=thrn, scalar2=None, op0=ALU.is_gt)
            selb = zwork.tile([P, ZW], f32, tag="selb")
            bandcnt = small.tile([P, 1], f32)
            nc.gpsimd.tensor_mul(selb, g01, inb_sb)
            nc.vector.tensor_reduce(
                out=bandcnt, in_=selb, axis=mybir.AxisListType.X, op=ALU.add)
            zjunk4 = zwork.tile([P, ZW], f32, tag="zjunk4")
            bs = small.tile([P, 1], f32)
            nc.gpsimd.tensor_mul(zjunk4, selb, simz)
            nc.vector.tensor_reduce(
                out=bs, in_=zjunk4, axis=mybir.AxisListType.X, op=ALU.add)
            s["bandcnt"], s["bs"] = bandcnt, bs

        def b_setup(mt):
            s = st[mt]
            s["racc"] = zwork.tile([P, HH], f32, tag="racc", name="racc")
            s["sacc"] = zwork.tile([P, HH], f32, tag="sacc", name="sacc")

        def b_tile(mt, h):
            s = st[mt]
            lhs = xtm_sb[:, mt * P:(mt + 1) * P]
            pb = psum.tile([P, PP], f32, tag="pp")
            for q_ in range(4):
                c0 = (4 * h + q_) * CH
                nc.tensor.matmul(
                    pb[:, q_ * CH:(q_ + 1) * CH], lhs,
                    xt_sb[:, c0:c0 + CH], start=True, stop=True)
            nc.scalar.activation(
                out=junk, in_=pb, func=ACTF.Relu, bias=s["nthrn"], scale=1.0,
                accum_out=s["racc"][:, h:h + 1])
            nc.scalar.activation(
                out=junk, in_=pb, func=ACTF.Sign, bias=s["nthrn"], scale=1.0,
                accum_out=s["sacc"][:, h:h + 1])

        def b_finalize(mt):
            s = st[mt]
            thrn = s["thrn"]
            relusum = small.tile([P, 1], f32)
            nc.vector.tensor_reduce(
                out=relusum, in_=s["racc"], axis=mybir.AxisListType.X, op=ALU.add)
            ssum = small.tile([P, 1], f32)
            nc.vector.tensor_reduce(
                out=ssum, in_=s["sacc"], axis=mybir.AxisListType.X, op=ALU.add)
            # cnt = (ssum + N) / 2   (sign is -1/+1; exact ties are measure-zero)
            cnt = small.tile([P, 1], f32)
            nc.vector.tensor_scalar(
                out=cnt, in0=ssum, scalar1=float(N), scalar2=0.5,
                op0=ALU.add, op1=ALU.mult)

            tmp1 = small.tile([P, 1], f32)
            nc.vector.tensor_mul(tmp1, thrn, cnt)
            negfull = small.tile([P, 1], f32)
            nc.vector.tensor_add(negfull, relusum, tmp1)
            negloss = small.tile([P, 1], f32)
            nc.vector.tensor_sub(negloss, negfull, s["bs"])
            posloss = small.tile([P, 1], f32)
            nc.vector.tensor_sub(posloss, s["cntp"], s["spsum"])
            dcnt = small.tile([P, 1], f32)
            nc.vector.tensor_sub(dcnt, cnt, s["bandcnt"])
            anyneg = small.tile([P, 1], f32)
            nc.vector.tensor_scalar(
                out=anyneg, in0=dcnt, scalar1=0.5, scalar2=None, op0=ALU.is_gt)
            tsum = small.tile([P, 1], f32)
            nc.vector.tensor_add(tsum, posloss, negloss)
            rl = small.tile([P, 1], f32)
            nc.vector.tensor_mul(rl, tsum, s["hp"])
            nr = small.tile([P, 1], f32)
            nc.vector.tensor_mul(nr, anyneg, s["hp"])
            nc.vector.tensor_add(accpair[:, 0:1], accpair[:, 0:1], rl)
            nc.vector.tensor_add(accpair[:, 1:2], accpair[:, 1:2], nr)

        ones = consts.tile([P, 1], f32)
        nc.vector.memset(ones, 1.0)

        def whole_pass():
            nc.vector.memset(accpair, 0.0)
            for mt in range(MT + LAG):
                if mt < MT:
                    a_setup(mt)
                if mt >= LAG:
                    b_setup(mt - LAG)
                for h in range(HH):
                    if mt >= LAG:
                        b_tile(mt - LAG, h)
                    if mt < MT:
                        a_tile(mt, h)
                if mt < MT:
                    a_chain(mt)
                if mt >= LAG:
                    b_finalize(mt - LAG)
            pfin = psum.tile([P, PP], f32, tag="pp")
            nc.tensor.matmul(pfin[0:1, 0:2], ones, accpair, start=True, stop=True)
            outsb = consts.tile([1, 2], f32)
            nc.scalar.copy(outsb, pfin[0:1, 0:2])
            nc.gpsimd.dma_start(out=out_d[:, :], in_=outsb)

        if bench_reps > 1:
            with tc.For_i(0, bench_reps, 1):
                whole_pass()
        else:
            whole_pass()

    nc.compile()
    return nc


def prep_inputs(x: np.ndarray, t: np.ndarray):
    """Sort rows by class, build per-core input maps."""
    perm = np.argsort(t, kind="stable")
    ts = t[perm]
    xs = np.ascontiguousarray(x[perm])
    xt = np.ascontiguousarray(xs.T.astype(np.float32))  # [D, N]

    change = np.r_[True, ts[1:] != ts[:-1]]
    grp = np.cumsum(change) - 1
    starts = np.flatnonzero(change)
    counts = np.diff(np.r_[starts, N])
    lo = starts[grp].astype(np.int64)
    hi = (starts[grp] + counts[grp]).astype(np.int64)
    haspos = (counts[grp] > 1).astype(np.float32)
    rows = np.arange(N, dtype=np.int64)

    in_maps = []
    for c in range(NCORES):
        r0c = c * RPC
        xtm = np.ascontiguousarray(xt[:, r0c:r0c + RPC])
        xtz = np.empty((MT, D, ZW), np.float32)
        inb = np.empty((MT, P, ZW), np.float32)
        posm = np.empty((MT, P, ZW), np.float32)
        adz = np.zeros((MT, NBLK), np.float32)
        hp = np.empty((MT, P, 1), np.float32)
        for mt in range(MT):
            r0 = r0c + mt * P
            LO = int(lo[r0])
            HI = int(hi[r0 + P - 1])
            z0 = min((LO // P) * P, N - ZW)
            assert HI <= z0 + ZW, (c, mt, LO, HI, z0)
            xtz[mt] = xt[:, z0:z0 + ZW]
            g = rows[r0:r0 + P]
            colg = z0 + np.arange(ZW, dtype=np.int64)
            band = (colg[None, :] >= lo[g][:, None]) & (colg[None, :] < hi[g][:, None])
            inb[mt] = band.astype(np.float32)
            posm[mt] = (band & (colg[None, :] != g[:, None])).astype(np.float32)
            adz[mt, z0 // P: z0 // P + 3] = -BIG
            hp[mt, :, 0] = haspos[g]
        bigin = np.concatenate(
            [xt, xtm, xtz.transpose(1, 0, 2).reshape(D, MT * ZW)], axis=1)
        maskz = np.concatenate([inb, posm, hp], axis=2)
        in_maps.append({
            "bigin": np.ascontiguousarray(bigin),
            "maskz": np.ascontiguousarray(maskz),
            "addz": adz,
        })
    return in_maps


_NC_CACHE = {}


def get_nc() -> bass.Bass:
    if "nc" not in _NC_CACHE:
        _NC_CACHE["nc"] = build_nc()
    return _NC_CACHE["nc"]


def kernel(inputs_col, targets_col, _trace=False, _trace_kwargs=None):
    x = np.asarray(inputs_col, dtype=np.float32)
    t = np.asarray(targets_col).astype(np.int64)
    assert x.shape == (N, D) and t.shape == (N,)

    in_maps = prep_inputs(x, t)
    nc = get_nc()
    kwargs = {}
    if _trace:
        kwargs["trace"] = True
        kwargs.update(_trace_kwargs or {})
    res = run_bass_kernel_spmd(nc, in_maps, core_ids=list(range(NCORES)), **kwargs)
    total = np.zeros(2, np.float64)
    for o in res.results:
        total += np.asarray(o["out"], np.float64)[0]
    loss = np.float32(np.float32(total[0]) / np.float32(N))
    neg_count = np.int32(np.rint(total[1]))
    if _trace:
        return (loss, neg_count), res
    return loss, neg_count

